# revision 8
# baseline (speedup 1.0000x reference)
"""Causal self-attention (B=2, T=2048, C=1024, H=16) on 8 TRN2 NeuronCores.

Sharding: core = b*4 + g  (b in 0..1 batches, g in 0..3 head-groups of 4 heads).
Each core computes QKV for its 4 heads (tensor-parallel columns of W_attn),
full causal attention over T=2048, and a partial projection
y_g @ W_proj[rows_g] -> [T, C].  Host sums the 4 partials per batch and adds
b_proj.

Device layout notes:
  - x is pre-transposed on host to xT [C, T] so all matmuls contract over
    partitions.
  - sim is computed transposed: simT[tk, tq] = k . q   (lhsT=kT, rhs=qT),
    exp via ScalarE (scale=1/sqrt(C) folded in), causal mask applied as a
    gpsimd affine_select (exact zeros) on the exp tiles.
  - attn@v: lhsT = [v | ones] per head -> psum rows 0..63 = unnormalized y^T,
    row 64 = softmax denominator.  DVE reciprocal -> tiny SBUF DMA to
    partition 0 -> gpsimd partition_broadcast yields a [64, 512] SBUF tile
    aligned with y^T; one DVE multiply evicts normalized y^T to SBUF bf16.
    (PE-broadcast fallback kept under use_pbcast=False.)
  - y^T is packed two heads per 128 partitions (odd heads placed via a small
    SBUF->SBUF DMA) so the projection contracts K=128.
  - all matmul inputs bf16 (host-cast), accumulation f32, partial outputs
    bf16 (summed in f32 on host).
"""

import sys

sys.path.insert(0, "/opt/trn_rl_repo")

import numpy as np
import ml_dtypes

BF16 = ml_dtypes.bfloat16

B, T, C = 2, 2048, 1024
H, D = 16, 64
HPC = 4          # heads per core
GC = HPC * D     # head-group channel width (256)
NT = T // 128    # 16 row tiles
NS = T // 512    # 4 query super-tiles

_cached = None


def _build(repeat=1, use_pbcast=True):
    # note: tensor_mul with both operands in PSUM is rejected by the walrus
    # BIR verifier, so the non-pbcast path uses a two-step evict.
    import concourse.bass as bass  # noqa: F401
    import concourse.mybir as mybir
    import concourse.tile as tile
    from concourse import bacc

    f32 = mybir.dt.float32
    bf16 = mybir.dt.bfloat16
    AF = mybir.ActivationFunctionType

    nc = bacc.Bacc(None, target_bir_lowering=False, debug=False)
    xt_d = nc.declare_dram_parameter("xt", [C, T], bf16, isOutput=False)
    wqk_d = nc.declare_dram_parameter("wqk", [C, 2 * GC], bf16, isOutput=False)
    wv_d = nc.declare_dram_parameter("wv", [C, GC], bf16, isOutput=False)
    wp_d = nc.declare_dram_parameter("wp", [GC, C], bf16, isOutput=False)
    out_d = nc.declare_dram_parameter("out", [T, C], bf16, isOutput=True)

    with tile.TileContext(nc) as tc:
        with (
            tc.tile_pool(name="const", bufs=1) as cpool,
            tc.tile_pool(name="exp", bufs=4) as epool,
            tc.tile_pool(name="ostg", bufs=2) as opool,
            tc.tile_pool(name="smal", bufs=3) as spool,
            tc.tile_pool(name="mm", bufs=2, space="PSUM") as mmp,
            tc.tile_pool(name="sim", bufs=2, space="PSUM") as simp_pool,
            tc.tile_pool(name="yp", bufs=2, space="PSUM") as ypp,
        ):
            yta_ref = [None]
            wp_ref = [None]

            def emit_once():
                # ---- inputs: weights first (small), then xT spread over
                # several DMAs ----
                wqk_re = wqk_d[:].rearrange("(c p) m -> p c m", p=128)
                wqka = cpool.tile([128, 4, 2 * GC], bf16, tag="wqka")
                nc.sync.dma_start(out=wqka[:], in_=wqk_re[:, 0:4, :])
                wqkb = cpool.tile([128, 4, 2 * GC], bf16, tag="wqkb")
                nc.sync.dma_start(out=wqkb[:], in_=wqk_re[:, 4:8, :])

                def wqk_slice(c, sl):
                    t_ = wqka if c < 4 else wqkb
                    return t_[:, c % 4, sl]
                # x^T split per query super-tile so s=0 compute starts after
                # ~1MB of DMA instead of the full 4MB; the first chunk is
                # further halved so the very first matmuls start sooner
                xt_parts = []
                xt_re = xt_d[:].rearrange("(c p) t -> p c t", p=128)
                for s in range(NS):
                    if s == 0:
                        xa = cpool.tile([128, 4, 512], bf16, tag="xt0a")
                        nc.sync.dma_start(out=xa[:], in_=xt_re[:, 0:4, 0:512])
                        xb = cpool.tile([128, 4, 512], bf16, tag="xt0b")
                        nc.sync.dma_start(out=xb[:], in_=xt_re[:, 4:8, 0:512])
                        xt_parts.append([(xa, 0), (xb, 4)])
                        wv = cpool.tile([128, 8, GC], bf16, tag="wv")
                        nc.sync.dma_start(
                            out=wv[:],
                            in_=wv_d[:].rearrange("(c p) m -> p c m", p=128),
                        )
                    else:
                        x_s = cpool.tile([128, 8, 512], bf16, tag=f"xt{s}")
                        nc.sync.dma_start(
                            out=x_s[:],
                            in_=xt_re[:, :, s * 512 : (s + 1) * 512],
                        )
                        xt_parts.append([(x_s, 0)])

                def xslice(s, c, sl):
                    for t_, c0 in xt_parts[s]:
                        if c0 <= c < c0 + 4 or (c0 == 0 and len(xt_parts[s]) == 1):
                            return t_[:, c - c0, sl]
                    raise AssertionError
                # W_proj rows for the head pair j live at partitions
                # [0..127] = channels j*128..j*128+127
                wp = cpool.tile([128, 2, C], bf16, tag="wp")
                wp_ref[0] = wp
                nc.sync.dma_start(
                    out=wp[:], in_=wp_d[:].rearrange("(j p) n -> p j n", p=128)
                )

                # ones row at partition 64 for the denominator broadcast
                ones64 = cpool.tile([65, 64], f32, tag="ones64")
                nc.any.memset(ones64[64:65, :], 1.0)
                zbias = cpool.tile([128, 1], f32, tag="zbias")
                nc.any.memset(zbias[:], 0.0)

                # ---- QKV tiles + attention, interleaved per query super-tile
                # so ScalarE exp work starts as early as possible ----
                qkT = cpool.tile([128, 4, T], bf16, tag="qkT")
                v1 = cpool.tile([128, NT, HPC, 65], bf16, tag="v1")
                nc.gpsimd.memset(v1[:, :, :, 64:65], 1.0)
                # y^T packed 2 heads per 128 partitions: [128, pair, T]
                yta = cpool.tile([128, 2, T], bf16, tag="yta")
                yta_ref[0] = yta

                for s in range(NS):
                    # q^T,k^T columns for this super-tile
                    for m in range(4):
                        ps = mmp.tile([128, 512], f32, tag="mm")
                        for c in range(8):
                            nc.tensor.matmul(
                                ps[:],
                                wqk_slice(c, slice(m * 128, (m + 1) * 128)),
                                xslice(s, c, slice(None)),
                                start=(c == 0),
                                stop=(c == 7),
                            )
                        nc.vector.tensor_copy(
                            qkT[:, m, s * 512 : (s + 1) * 512], ps[:]
                        )
                    # v rows for this super-tile (+ ones column)
                    for t in range(s * 4, s * 4 + 4):
                        ps = mmp.tile([128, GC], f32, tag="mm")
                        for c in range(8):
                            nc.tensor.matmul(
                                ps[:],
                                xslice(
                                    s,
                                    c,
                                    slice((t - 4 * s) * 128, (t - 4 * s + 1) * 128),
                                ),
                                wv[:, c, :],
                                start=(c == 0),
                                stop=(c == 7),
                            )
                        nc.vector.tensor_copy(
                            v1[:, t, :, 0:64],
                            ps[:].rearrange("p (l d) -> p l d", d=64),
                        )

                    for li, l in enumerate((0, 1, 3, 2)):
                        if s > 0 and li in (1, 2):
                            # projection for the previous super-tile, deferred
                            # and split across two heads so its y^T inputs
                            # (incl. the odd-head SBUF DMA) have settled and
                            # the mm-psum pool isn't hammered all at once
                            emit_proj(s - 1, (0, 1) if li == 1 else (2, 3))
                        poff = (l % 2) * 64
                        qt = l // 2
                        q_ap = qkT[poff : poff + 64, qt, s * 512 : (s + 1) * 512]
                        njt = 4 * (s + 1)
                        yps = ypp.tile([65, 512], f32, tag="y")
                        for grp in range(njt // 2):
                            sp = simp_pool.tile([128, 1024], f32, tag="sim")
                            for jj in range(2):
                                j = grp * 2 + jj
                                r = j - 4 * s
                                # causal: diagonal tile j=4s+r only needs
                                # query columns >= r*128
                                q0 = r * 128 if r > 0 else 0
                                k_ap = qkT[
                                    poff : poff + 64, 2 + qt, j * 128 : (j + 1) * 128
                                ]
                                nc.tensor.matmul(
                                    sp[:, jj * 512 + q0 : (jj + 1) * 512],
                                    k_ap,
                                    q_ap[:, q0:],
                                    start=True,
                                    stop=True,
                                )
                            ex = epool.tile([128, 1024], bf16, tag="exp")
                            if grp >= 2 * s:
                                # diagonal pair: exp only the causally valid
                                # column range of each tile
                                for jj in range(2):
                                    j = grp * 2 + jj
                                    r = j - 4 * s
                                    q0 = r * 128 if r > 0 else 0
                                    nc.scalar.activation(
                                        ex[:, jj * 512 + q0 : (jj + 1) * 512],
                                        sp[:, jj * 512 + q0 : (jj + 1) * 512],
                                        AF.Exp,
                                        bias=zbias[:, 0:1],
                                        scale=1.0 / 32.0,
                                    )
                            else:
                                nc.scalar.activation(
                                    ex[:],
                                    sp[:],
                                    AF.Exp,
                                    bias=zbias[:, 0:1],
                                    scale=1.0 / 32.0,
                                )
                            for jj in range(2):
                                j = grp * 2 + jj
                                r = j - 4 * s
                                q0 = r * 128 if r > 0 else 0
                                if 0 <= r < 4:
                                    # zero below-diagonal within the 128-col
                                    # diagonal block: keep where f' >= p.
                                    # Columns past the block are fully valid.
                                    nc.gpsimd.affine_select(
                                        out=ex[:, jj * 512 + q0 : jj * 512 + q0 + 128],
                                        in_=ex[:, jj * 512 + q0 : jj * 512 + q0 + 128],
                                        pattern=[[1, 128]],
                                        compare_op=mybir.AluOpType.is_ge,
                                        fill=0.0,
                                        base=0,
                                        channel_multiplier=-1,
                                    )
                                nc.tensor.matmul(
                                    yps[:, q0:],
                                    v1[:, j, l, :],
                                    ex[:, jj * 512 + q0 : (jj + 1) * 512],
                                    start=(j == 0),
                                    stop=(j == njt - 1),
                                    skip_group_check=True,
                                )
                        # normalize: row 64 of yps is the denominator
                        rt = spool.tile([65, 512], f32, tag="rt")
                        nc.vector.reciprocal(rt[64:65, :], yps[64:65, :])
                        if l % 2 == 0:
                            ysl = yta[0:64, qt, s * 512 : (s + 1) * 512]
                        else:
                            ytmp = spool.tile([64, 512], bf16, tag="ytmp")
                            ysl = ytmp[:]
                        # for the very last head the PE is idle anyway and
                        # the engine-local PE-broadcast chain avoids the DMA
                        # hop latency right before the tail projection
                        if use_pbcast and not (s == NS - 1 and li == 3):
                            # partition_broadcast on HW reads the tile's
                            # physical partition 0 - hop the denominator row
                            # down with a tiny SBUF->SBUF DMA first
                            rt0 = spool.tile([1, 512], f32, tag="rt0")
                            nc.sync.dma_start(out=rt0[:], in_=rt[64:65, :])
                            bps = spool.tile([64, 512], f32, tag="bps")
                            nc.gpsimd.partition_broadcast(bps[:], rt0[:])
                            nc.vector.tensor_mul(ysl, yps[0:64, :], bps[:])
                        else:
                            bp = mmp.tile([64, 512], f32, tag="mm")
                            nc.tensor.matmul(
                                bp[:],
                                ones64[64:65, :],
                                rt[64:65, :],
                                start=True,
                                stop=True,
                            )
                            nc.vector.tensor_copy(ysl, yps[0:64, :])
                            nc.vector.tensor_mul(ysl, ysl, bp[:])
                        if l % 2 == 1:
                            nc.sync.dma_start(
                                out=yta[64:128, qt, s * 512 : (s + 1) * 512],
                                in_=ysl,
                            )

                # last super-tile's projection runs at the tail
                emit_proj(NS - 1, (0, 1, 2, 3))

            def emit_proj(s, tts):
                for tt in tts:
                    t = s * 4 + tt
                    ost = opool.tile([128, C], bf16, tag="ost")
                    for n in range(2):
                        pp = mmp.tile([128, 512], f32, tag="mm")
                        for j in range(2):
                            nc.tensor.matmul(
                                pp[:],
                                yta_ref[0][:, j, t * 128 : (t + 1) * 128],
                                wp_ref[0][:, j, n * 512 : (n + 1) * 512],
                                start=(j == 0),
                                stop=(j == 1),
                            )
                        nc.vector.tensor_copy(ost[:, n * 512 : (n + 1) * 512], pp[:])
                    nc.sync.dma_start(
                        out=out_d[t * 128 : (t + 1) * 128, :], in_=ost[:]
                    )

            for _rep in range(repeat):
                emit_once()

    nc.compile()
    return nc


def _get_nc():
    global _cached
    if _cached is None:
        _cached = _build()
    return _cached


def build_in_maps(inputs):
    # b_attn is spec-guaranteed zeros and is not used on-device; b_proj is
    # added on host.
    x = np.asarray(inputs["x"], dtype=np.float32)
    W_attn = np.asarray(inputs["W_attn"], dtype=np.float32)
    W_proj = np.asarray(inputs["W_proj"], dtype=np.float32)

    in_maps = []
    for b in range(B):
        xT = np.ascontiguousarray(x[b].T).astype(BF16)
        for g in range(4):
            c0 = g * GC
            wq = W_attn[:, c0 : c0 + GC]
            wk = W_attn[:, C + c0 : C + c0 + GC]
            wqk = np.ascontiguousarray(np.concatenate([wq, wk], axis=1)).astype(BF16)
            wv = np.ascontiguousarray(
                W_attn[:, 2 * C + c0 : 2 * C + c0 + GC]
            ).astype(BF16)
            wp = np.ascontiguousarray(W_proj[c0 : c0 + GC, :]).astype(BF16)
            in_maps.append({"xt": xT, "wqk": wqk, "wv": wv, "wp": wp})
    return in_maps


def kernel(x, W_attn, b_attn, W_proj, b_proj):
    from concourse.bass_utils import run_bass_kernel_spmd

    b_proj = np.asarray(b_proj, dtype=np.float32)
    nc = _get_nc()
    in_maps = build_in_maps(
        {"x": x, "W_attn": W_attn, "b_attn": b_attn, "W_proj": W_proj}
    )
    res = run_bass_kernel_spmd(nc, in_maps, core_ids=list(range(8)))
    out = np.zeros((B, T, C), dtype=np.float32)
    for b in range(B):
        for g in range(4):
            out[b] += res.results[b * 4 + g]["out"].astype(np.float32)
        out[b] += b_proj
    return out



# revision 22
# speedup vs baseline: 8.2420x; 8.2420x over previous
"""Causal self-attention (B=2, T=2048, C=1024, H=16) on 8 TRN2 NeuronCores.

Sharding: core = b*4 + g  (b in 0..1 batches, g in 0..3 head-groups of 4 heads).
Each core computes QKV for its 4 heads (tensor-parallel columns of W_attn),
full causal attention over T=2048, and a partial projection
y_g @ W_proj[rows_g] -> [T, C].  Host sums the 4 partials per batch and adds
b_proj.

Device layout notes:
  - x is pre-transposed on host to xT [C, T] so all matmuls contract over
    partitions.
  - sim is computed transposed: simT[tk, tq] = k . q   (lhsT=kT, rhs=qT),
    exp via ScalarE (scale=1/sqrt(C) folded in), causal mask applied as a
    gpsimd affine_select (exact zeros) on the exp tiles.
  - attn@v: lhsT = [v | ones] per head -> psum rows 0..63 = unnormalized y^T,
    row 64 = softmax denominator.  DVE reciprocal -> tiny SBUF DMA to
    partition 0 -> gpsimd partition_broadcast yields a [64, 512] SBUF tile
    aligned with y^T; one DVE multiply evicts normalized y^T to SBUF bf16.
    (PE-broadcast fallback kept under use_pbcast=False.)
  - y^T is packed two heads per 128 partitions (odd heads placed via a small
    SBUF->SBUF DMA) so the projection contracts K=128.
  - all matmul inputs bf16 (host-cast), accumulation f32, partial outputs
    bf16 (summed in f32 on host).
"""

import sys

sys.path.insert(0, "/opt/trn_rl_repo")

import numpy as np
import ml_dtypes

BF16 = ml_dtypes.bfloat16

B, T, C = 2, 2048, 1024
H, D = 16, 64
HPC = 4          # heads per core
GC = HPC * D     # head-group channel width (256)
NT = T // 128    # 16 row tiles
NS = T // 512    # 4 query super-tiles

FP8_SIM = True   # k.q sim matmuls in fp8e4 DoubleRow (zero-padded slot 1)
FP8_QK = True    # q,k QKV matmuls in fp8e4 DoubleRow (paired c-tiles)

_cached = None


def _build(repeat=1, use_pbcast=True, fp8_sim=FP8_SIM, fp8_qk=FP8_QK):
    # note: tensor_mul with both operands in PSUM is rejected by the walrus
    # BIR verifier, so the non-pbcast path uses a two-step evict.
    import concourse.bass as bass  # noqa: F401
    import concourse.mybir as mybir
    import concourse.tile as tile
    from concourse import bacc

    f32 = mybir.dt.float32
    bf16 = mybir.dt.bfloat16
    f8 = mybir.dt.float8e4
    AF = mybir.ActivationFunctionType
    DR = mybir.MatmulPerfMode.DoubleRow

    nc = bacc.Bacc(None, target_bir_lowering=False, debug=False)
    xt_d = nc.declare_dram_parameter("xt", [C, T], bf16, isOutput=False)
    if fp8_qk:
        xt8_d = nc.declare_dram_parameter("xt8", [C, T], f8, isOutput=False)
        wqk8_d = nc.declare_dram_parameter(
            "wqk8", [C, 2 * GC], f8, isOutput=False
        )
    else:
        wqk_d = nc.declare_dram_parameter(
            "wqk", [C, 2 * GC], bf16, isOutput=False
        )
    wv_d = nc.declare_dram_parameter("wv", [C, GC], bf16, isOutput=False)
    wp_d = nc.declare_dram_parameter("wp", [GC, C], bf16, isOutput=False)
    out_d = nc.declare_dram_parameter("out", [T, C], bf16, isOutput=True)

    with tile.TileContext(nc) as tc:
        with (
            tc.tile_pool(name="const", bufs=1) as cpool,
            tc.tile_pool(name="exp", bufs=4) as epool,
            tc.tile_pool(name="ostg", bufs=2) as opool,
            tc.tile_pool(name="smal", bufs=3) as spool,
            tc.tile_pool(name="mm", bufs=2, space="PSUM") as mmp,
            tc.tile_pool(name="sim", bufs=2, space="PSUM") as simp_pool,
            tc.tile_pool(name="yp", bufs=2, space="PSUM") as ypp,
        ):
            yta_ref = [None]
            wp_ref = [None]

            def emit_once():
                # ---- inputs: weights first (small), then xT spread over
                # several DMAs ----
                if fp8_qk:
                    # paired-c-tile fp8 layout for DoubleRow: [p, u, i, cols],
                    # contraction c = (2u+i)*128 + p
                    wqk8 = cpool.tile([128, 4, 2, 2 * GC], f8, tag="wqk8")
                    nc.sync.dma_start(
                        out=wqk8[:],
                        in_=wqk8_d[:].rearrange(
                            "(u i p) m -> p u i m", p=128, i=2
                        ),
                    )
                    xt8_re = xt8_d[:].rearrange("(u i p) t -> p u i t", p=128, i=2)
                    x8a = cpool.tile([128, 2, 2, 512], f8, tag="x8_0a")
                    nc.sync.dma_start(out=x8a[:], in_=xt8_re[:, 0:2, :, 0:512])
                    x8b = cpool.tile([128, 2, 2, 512], f8, tag="x8_0b")
                    nc.sync.dma_start(out=x8b[:], in_=xt8_re[:, 2:4, :, 0:512])
                    x8_parts = [[(x8a, 0), (x8b, 2)]]
                    for s in range(1, NS):
                        x8s = cpool.tile([128, 4, 2, 512], f8, tag=f"x8_{s}")
                        nc.sync.dma_start(
                            out=x8s[:],
                            in_=xt8_re[:, :, :, s * 512 : (s + 1) * 512],
                        )
                        x8_parts.append([(x8s, 0)])

                    def x8slice(s, u):
                        for t_, u0 in x8_parts[s]:
                            if u0 <= u < u0 + 2 or (
                                u0 == 0 and len(x8_parts[s]) == 1
                            ):
                                return t_[:, u - u0, :, :]
                        raise AssertionError
                else:
                    wqk_re = wqk_d[:].rearrange("(c p) m -> p c m", p=128)
                    wqka = cpool.tile([128, 4, 2 * GC], bf16, tag="wqka")
                    nc.sync.dma_start(out=wqka[:], in_=wqk_re[:, 0:4, :])
                    wqkb = cpool.tile([128, 4, 2 * GC], bf16, tag="wqkb")
                    nc.sync.dma_start(out=wqkb[:], in_=wqk_re[:, 4:8, :])

                    def wqk_slice(c, sl):
                        t_ = wqka if c < 4 else wqkb
                        return t_[:, c % 4, sl]
                # x^T split per query super-tile so s=0 compute starts after
                # ~1MB of DMA instead of the full 4MB; the first chunk is
                # further halved so the very first matmuls start sooner
                xt_parts = []
                xt_re = xt_d[:].rearrange("(c p) t -> p c t", p=128)
                for s in range(NS):
                    if s == 0:
                        xa = cpool.tile([128, 4, 512], bf16, tag="xt0a")
                        nc.sync.dma_start(out=xa[:], in_=xt_re[:, 0:4, 0:512])
                        xb = cpool.tile([128, 4, 512], bf16, tag="xt0b")
                        nc.sync.dma_start(out=xb[:], in_=xt_re[:, 4:8, 0:512])
                        xt_parts.append([(xa, 0), (xb, 4)])
                        wv = cpool.tile([128, 8, GC], bf16, tag="wv")
                        nc.sync.dma_start(
                            out=wv[:],
                            in_=wv_d[:].rearrange("(c p) m -> p c m", p=128),
                        )
                    else:
                        x_s = cpool.tile([128, 8, 512], bf16, tag=f"xt{s}")
                        nc.sync.dma_start(
                            out=x_s[:],
                            in_=xt_re[:, :, s * 512 : (s + 1) * 512],
                        )
                        xt_parts.append([(x_s, 0)])

                def xslice(s, c, sl):
                    for t_, c0 in xt_parts[s]:
                        if c0 <= c < c0 + 4 or (c0 == 0 and len(xt_parts[s]) == 1):
                            return t_[:, c - c0, sl]
                    raise AssertionError
                # W_proj rows for the head pair j live at partitions
                # [0..127] = channels j*128..j*128+127
                wp = cpool.tile([128, 2, C], bf16, tag="wp")
                wp_ref[0] = wp
                nc.sync.dma_start(
                    out=wp[:], in_=wp_d[:].rearrange("(j p) n -> p j n", p=128)
                )

                # ones row at partition 64 for the denominator broadcast
                ones64 = cpool.tile([65, 64], f32, tag="ones64")
                nc.any.memset(ones64[64:65, :], 1.0)
                zbias = cpool.tile([128, 1], f32, tag="zbias")
                nc.any.memset(zbias[:], 0.0)

                # ---- QKV tiles + attention, interleaved per query super-tile
                # so ScalarE exp work starts as early as possible ----
                if fp8_sim:
                    # q,k quantized to fp8e4 in [128, slot, hh, T] layout for
                    # DoubleRow sim matmuls; slot 1 is all-zero padding (the
                    # pairing contributes k.q + 0.junk), so evictions stay
                    # one [128,512] copy per m-tile and no partition moves
                    # are needed.  Head h=2*hh+a lives at partitions 64a..64a+63.
                    qz8 = cpool.tile([128, 2, 2, T], f8, tag="qz8")
                    kz8 = cpool.tile([128, 2, 2, T], f8, tag="kz8")
                    nc.gpsimd.memset(qz8[:, 1, :, :], 0.0)
                    nc.gpsimd.memset(kz8[:, 1, :, :], 0.0)
                else:
                    qkT = cpool.tile([128, 4, T], bf16, tag="qkT")
                v1 = cpool.tile([128, NT, HPC, 65], bf16, tag="v1")
                nc.gpsimd.memset(v1[:, :, :, 64:65], 1.0)
                # y^T packed 2 heads per 128 partitions: [128, pair, T]
                yta = cpool.tile([128, 2, T], bf16, tag="yta")
                yta_ref[0] = yta

                for s in range(NS):
                    # q^T,k^T columns for this super-tile
                    for m in range(4):
                        ps = mmp.tile([128, 512], f32, tag="mm")
                        if fp8_qk:
                            for u in range(4):
                                nc.tensor.matmul(
                                    ps[:],
                                    wqk8[:, u, :, m * 128 : (m + 1) * 128],
                                    x8slice(s, u),
                                    start=(u == 0),
                                    stop=(u == 3),
                                    perf_mode=DR,
                                )
                        else:
                            for c in range(8):
                                nc.tensor.matmul(
                                    ps[:],
                                    wqk_slice(c, slice(m * 128, (m + 1) * 128)),
                                    xslice(s, c, slice(None)),
                                    start=(c == 0),
                                    stop=(c == 7),
                                )
                        if fp8_sim:
                            dst = qz8 if m < 2 else kz8
                            nc.vector.tensor_copy(
                                dst[:, 0, m % 2, s * 512 : (s + 1) * 512], ps[:]
                            )
                        else:
                            nc.vector.tensor_copy(
                                qkT[:, m, s * 512 : (s + 1) * 512], ps[:]
                            )
                    # v rows for this super-tile (+ ones column)
                    for t in range(s * 4, s * 4 + 4):
                        ps = mmp.tile([128, GC], f32, tag="mm")
                        for c in range(8):
                            nc.tensor.matmul(
                                ps[:],
                                xslice(
                                    s,
                                    c,
                                    slice((t - 4 * s) * 128, (t - 4 * s + 1) * 128),
                                ),
                                wv[:, c, :],
                                start=(c == 0),
                                stop=(c == 7),
                            )
                        nc.vector.tensor_copy(
                            v1[:, t, :, 0:64],
                            ps[:].rearrange("p (l d) -> p l d", d=64),
                        )

                    for li, l in enumerate((0, 1, 3, 2)):
                        if s > 0 and li in (1, 2):
                            # projection for the previous super-tile, deferred
                            # and split across two heads so its y^T inputs
                            # (incl. the odd-head SBUF DMA) have settled and
                            # the mm-psum pool isn't hammered all at once
                            emit_proj(s - 1, (0, 1) if li == 1 else (2, 3))
                        poff = (l % 2) * 64
                        qt = l // 2
                        if not fp8_sim:
                            q_ap = qkT[poff : poff + 64, qt, s * 512 : (s + 1) * 512]
                        njt = 4 * (s + 1)
                        yps = ypp.tile([65, 512], f32, tag="y")
                        for grp in range(njt // 2):
                            sp = simp_pool.tile([128, 1024], f32, tag="sim")
                            for jj in range(2):
                                j = grp * 2 + jj
                                r = j - 4 * s
                                # causal: diagonal tile j=4s+r only needs
                                # query columns >= r*128
                                q0 = r * 128 if r > 0 else 0
                                if fp8_sim:
                                    nc.tensor.matmul(
                                        sp[:, jj * 512 + q0 : (jj + 1) * 512],
                                        kz8[
                                            poff : poff + 64,
                                            :,
                                            qt,
                                            j * 128 : (j + 1) * 128,
                                        ],
                                        qz8[
                                            poff : poff + 64,
                                            :,
                                            qt,
                                            s * 512 + q0 : (s + 1) * 512,
                                        ],
                                        start=True,
                                        stop=True,
                                        perf_mode=DR,
                                    )
                                else:
                                    k_ap = qkT[
                                        poff : poff + 64,
                                        2 + qt,
                                        j * 128 : (j + 1) * 128,
                                    ]
                                    nc.tensor.matmul(
                                        sp[:, jj * 512 + q0 : (jj + 1) * 512],
                                        k_ap,
                                        q_ap[:, q0:],
                                        start=True,
                                        stop=True,
                                    )
                            ex = epool.tile([128, 1024], bf16, tag="exp")
                            if grp >= 2 * s:
                                # diagonal pair: exp only the causally valid
                                # column range of each tile
                                for jj in range(2):
                                    j = grp * 2 + jj
                                    r = j - 4 * s
                                    q0 = r * 128 if r > 0 else 0
                                    nc.scalar.activation(
                                        ex[:, jj * 512 + q0 : (jj + 1) * 512],
                                        sp[:, jj * 512 + q0 : (jj + 1) * 512],
                                        AF.Exp,
                                        bias=zbias[:, 0:1],
                                        scale=1.0 / 32.0,
                                    )
                            else:
                                nc.scalar.activation(
                                    ex[:],
                                    sp[:],
                                    AF.Exp,
                                    bias=zbias[:, 0:1],
                                    scale=1.0 / 32.0,
                                )
                            for jj in range(2):
                                j = grp * 2 + jj
                                r = j - 4 * s
                                q0 = r * 128 if r > 0 else 0
                                if 0 <= r < 4:
                                    # zero below-diagonal within the 128-col
                                    # diagonal block: keep where f' >= p.
                                    # Columns past the block are fully valid.
                                    nc.gpsimd.affine_select(
                                        out=ex[:, jj * 512 + q0 : jj * 512 + q0 + 128],
                                        in_=ex[:, jj * 512 + q0 : jj * 512 + q0 + 128],
                                        pattern=[[1, 128]],
                                        compare_op=mybir.AluOpType.is_ge,
                                        fill=0.0,
                                        base=0,
                                        channel_multiplier=-1,
                                    )
                                nc.tensor.matmul(
                                    yps[:, q0:],
                                    v1[:, j, l, :],
                                    ex[:, jj * 512 + q0 : (jj + 1) * 512],
                                    start=(j == 0),
                                    stop=(j == njt - 1),
                                    skip_group_check=True,
                                )
                        # normalize: row 64 of yps is the denominator
                        rt = spool.tile([65, 512], f32, tag="rt")
                        nc.vector.reciprocal(rt[64:65, :], yps[64:65, :])
                        if l % 2 == 0:
                            ysl = yta[0:64, qt, s * 512 : (s + 1) * 512]
                        else:
                            ytmp = spool.tile([64, 512], bf16, tag="ytmp")
                            ysl = ytmp[:]
                        # for the very last head the PE is idle anyway and
                        # the engine-local PE-broadcast chain avoids the DMA
                        # hop latency right before the tail projection
                        if use_pbcast and not (s == NS - 1 and li == 3):
                            # partition_broadcast on HW reads the tile's
                            # physical partition 0 - hop the denominator row
                            # down with a tiny SBUF->SBUF DMA first
                            rt0 = spool.tile([1, 512], f32, tag="rt0")
                            nc.sync.dma_start(out=rt0[:], in_=rt[64:65, :])
                            bps = spool.tile([64, 512], f32, tag="bps")
                            nc.gpsimd.partition_broadcast(bps[:], rt0[:])
                            nc.vector.tensor_mul(ysl, yps[0:64, :], bps[:])
                        else:
                            bp = mmp.tile([64, 512], f32, tag="mm")
                            nc.tensor.matmul(
                                bp[:],
                                ones64[64:65, :],
                                rt[64:65, :],
                                start=True,
                                stop=True,
                            )
                            nc.vector.tensor_copy(ysl, yps[0:64, :])
                            nc.vector.tensor_mul(ysl, ysl, bp[:])
                        if l % 2 == 1:
                            nc.sync.dma_start(
                                out=yta[64:128, qt, s * 512 : (s + 1) * 512],
                                in_=ysl,
                            )

                # last super-tile's projection runs at the tail
                emit_proj(NS - 1, (0, 1, 2, 3))

            def emit_proj(s, tts):
                for tt in tts:
                    t = s * 4 + tt
                    ost = opool.tile([128, C], bf16, tag="ost")
                    for n in range(2):
                        pp = mmp.tile([128, 512], f32, tag="mm")
                        for j in range(2):
                            nc.tensor.matmul(
                                pp[:],
                                yta_ref[0][:, j, t * 128 : (t + 1) * 128],
                                wp_ref[0][:, j, n * 512 : (n + 1) * 512],
                                start=(j == 0),
                                stop=(j == 1),
                            )
                        nc.vector.tensor_copy(ost[:, n * 512 : (n + 1) * 512], pp[:])
                    nc.sync.dma_start(
                        out=out_d[t * 128 : (t + 1) * 128, :], in_=ost[:]
                    )

            for _rep in range(repeat):
                emit_once()

    nc.compile()
    return nc


def _get_nc():
    global _cached
    if _cached is None:
        _cached = _build()
    return _cached


def build_in_maps(inputs):
    # b_attn is spec-guaranteed zeros and is not used on-device; b_proj is
    # added on host.
    x = np.asarray(inputs["x"], dtype=np.float32)
    W_attn = np.asarray(inputs["W_attn"], dtype=np.float32)
    W_proj = np.asarray(inputs["W_proj"], dtype=np.float32)

    F8 = ml_dtypes.float8_e4m3
    in_maps = []
    for b in range(B):
        xT = np.ascontiguousarray(x[b].T).astype(BF16)
        xT8 = xT.astype(F8) if FP8_QK else None
        for g in range(4):
            c0 = g * GC
            wq = W_attn[:, c0 : c0 + GC]
            wk = W_attn[:, C + c0 : C + c0 + GC]
            wqk = np.ascontiguousarray(np.concatenate([wq, wk], axis=1)).astype(BF16)
            wv = np.ascontiguousarray(
                W_attn[:, 2 * C + c0 : 2 * C + c0 + GC]
            ).astype(BF16)
            wp = np.ascontiguousarray(W_proj[c0 : c0 + GC, :]).astype(BF16)
            m = {"xt": xT, "wv": wv, "wp": wp}
            if FP8_QK:
                m["xt8"] = xT8
                m["wqk8"] = wqk.astype(F8)
            else:
                m["wqk"] = wqk
            in_maps.append(m)
    return in_maps


def kernel(x, W_attn, b_attn, W_proj, b_proj):
    from concourse.bass_utils import run_bass_kernel_spmd

    b_proj = np.asarray(b_proj, dtype=np.float32)
    nc = _get_nc()
    in_maps = build_in_maps(
        {"x": x, "W_attn": W_attn, "b_attn": b_attn, "W_proj": W_proj}
    )
    res = run_bass_kernel_spmd(nc, in_maps, core_ids=list(range(8)))
    out = np.zeros((B, T, C), dtype=np.float32)
    for b in range(B):
        for g in range(4):
            out[b] += res.results[b * 4 + g]["out"].astype(np.float32)
        out[b] += b_proj
    return out



# revision 23
# speedup vs baseline: 11.0486x; 1.3405x over previous
"""Causal self-attention (B=2, T=2048, C=1024, H=16) on 8 TRN2 NeuronCores.

Sharding: core = b*4 + g  (b in 0..1 batches, g in 0..3 head-groups of 4 heads).
Each core computes QKV for its 4 heads (tensor-parallel columns of W_attn),
full causal attention over T=2048, and a partial projection
y_g @ W_proj[rows_g] -> [T, C].  Host sums the 4 partials per batch and adds
b_proj.

Device layout notes:
  - x is pre-transposed on host to xT [C, T] so all matmuls contract over
    partitions.
  - sim is computed transposed: simT[tk, tq] = k . q   (lhsT=kT, rhs=qT),
    exp via ScalarE (scale=1/sqrt(C) folded in), causal mask applied as a
    gpsimd affine_select (exact zeros) on the exp tiles.
  - attn@v: lhsT = [v | ones] per head -> psum rows 0..63 = unnormalized y^T,
    row 64 = softmax denominator.  DVE reciprocal -> tiny SBUF DMA to
    partition 0 -> gpsimd partition_broadcast yields a [64, 512] SBUF tile
    aligned with y^T; one DVE multiply evicts normalized y^T to SBUF bf16.
    (PE-broadcast fallback kept under use_pbcast=False.)
  - y^T is packed two heads per 128 partitions (odd heads placed via a small
    SBUF->SBUF DMA) so the projection contracts K=128.
  - all matmul inputs bf16 (host-cast), accumulation f32, partial outputs
    bf16 (summed in f32 on host).
"""

import sys

sys.path.insert(0, "/opt/trn_rl_repo")

import numpy as np
import ml_dtypes

BF16 = ml_dtypes.bfloat16

B, T, C = 2, 2048, 1024
H, D = 16, 64
HPC = 4          # heads per core
GC = HPC * D     # head-group channel width (256)
NT = T // 128    # 16 row tiles
NS = T // 512    # 4 query super-tiles

FP8_SIM = True   # k.q sim matmuls in fp8e4 DoubleRow (zero-padded slot 1)
FP8_QK = False   # q,k QKV matmuls in fp8e4 DoubleRow (paired c-tiles)

_cached = None


def _build(repeat=1, use_pbcast=True, fp8_sim=FP8_SIM, fp8_qk=FP8_QK):
    # note: tensor_mul with both operands in PSUM is rejected by the walrus
    # BIR verifier, so the non-pbcast path uses a two-step evict.
    import concourse.bass as bass  # noqa: F401
    import concourse.mybir as mybir
    import concourse.tile as tile
    from concourse import bacc

    f32 = mybir.dt.float32
    bf16 = mybir.dt.bfloat16
    f8 = mybir.dt.float8e4
    AF = mybir.ActivationFunctionType
    DR = mybir.MatmulPerfMode.DoubleRow

    nc = bacc.Bacc(None, target_bir_lowering=False, debug=False)
    xt_d = nc.declare_dram_parameter("xt", [C, T], bf16, isOutput=False)
    if fp8_qk:
        xt8_d = nc.declare_dram_parameter("xt8", [C, T], f8, isOutput=False)
        wqk8_d = nc.declare_dram_parameter(
            "wqk8", [C, 2 * GC], f8, isOutput=False
        )
    else:
        wqk_d = nc.declare_dram_parameter(
            "wqk", [C, 2 * GC], bf16, isOutput=False
        )
    wv_d = nc.declare_dram_parameter("wv", [C, GC], bf16, isOutput=False)
    wp_d = nc.declare_dram_parameter("wp", [GC, C], bf16, isOutput=False)
    out_d = nc.declare_dram_parameter("out", [T, C], bf16, isOutput=True)

    with tile.TileContext(nc) as tc:
        with (
            tc.tile_pool(name="const", bufs=1) as cpool,
            tc.tile_pool(name="exp", bufs=4) as epool,
            tc.tile_pool(name="ostg", bufs=2) as opool,
            tc.tile_pool(name="smal", bufs=3) as spool,
            tc.tile_pool(name="mm", bufs=2, space="PSUM") as mmp,
            tc.tile_pool(name="sim", bufs=2, space="PSUM") as simp_pool,
            tc.tile_pool(name="yp", bufs=2, space="PSUM") as ypp,
        ):
            yta_ref = [None]
            wp_ref = [None]

            def emit_once():
                # ---- inputs: weights first (small), then xT spread over
                # several DMAs ----
                if fp8_qk:
                    # paired-c-tile fp8 layout for DoubleRow: [p, u, i, cols],
                    # contraction c = (2u+i)*128 + p
                    wqk8 = cpool.tile([128, 4, 2, 2 * GC], f8, tag="wqk8")
                    nc.sync.dma_start(
                        out=wqk8[:],
                        in_=wqk8_d[:].rearrange(
                            "(u i p) m -> p u i m", p=128, i=2
                        ),
                    )
                    xt8_re = xt8_d[:].rearrange("(u i p) t -> p u i t", p=128, i=2)
                    x8a = cpool.tile([128, 2, 2, 512], f8, tag="x8_0a")
                    nc.sync.dma_start(out=x8a[:], in_=xt8_re[:, 0:2, :, 0:512])
                    x8b = cpool.tile([128, 2, 2, 512], f8, tag="x8_0b")
                    nc.sync.dma_start(out=x8b[:], in_=xt8_re[:, 2:4, :, 0:512])
                    x8_parts = [[(x8a, 0), (x8b, 2)]]
                    for s in range(1, NS):
                        x8s = cpool.tile([128, 4, 2, 512], f8, tag=f"x8_{s}")
                        nc.sync.dma_start(
                            out=x8s[:],
                            in_=xt8_re[:, :, :, s * 512 : (s + 1) * 512],
                        )
                        x8_parts.append([(x8s, 0)])

                    def x8slice(s, u):
                        for t_, u0 in x8_parts[s]:
                            if u0 <= u < u0 + 2 or (
                                u0 == 0 and len(x8_parts[s]) == 1
                            ):
                                return t_[:, u - u0, :, :]
                        raise AssertionError
                else:
                    wqk_re = wqk_d[:].rearrange("(c p) m -> p c m", p=128)
                    wqka = cpool.tile([128, 4, 2 * GC], bf16, tag="wqka")
                    nc.sync.dma_start(out=wqka[:], in_=wqk_re[:, 0:4, :])
                    wqkb = cpool.tile([128, 4, 2 * GC], bf16, tag="wqkb")
                    nc.sync.dma_start(out=wqkb[:], in_=wqk_re[:, 4:8, :])

                    def wqk_slice(c, sl):
                        t_ = wqka if c < 4 else wqkb
                        return t_[:, c % 4, sl]
                # x^T split per query super-tile so s=0 compute starts after
                # ~1MB of DMA instead of the full 4MB; the first chunk is
                # further halved so the very first matmuls start sooner
                xt_parts = []
                xt_re = xt_d[:].rearrange("(c p) t -> p c t", p=128)
                for s in range(NS):
                    if s == 0:
                        xa = cpool.tile([128, 4, 512], bf16, tag="xt0a")
                        nc.sync.dma_start(out=xa[:], in_=xt_re[:, 0:4, 0:512])
                        xb = cpool.tile([128, 4, 512], bf16, tag="xt0b")
                        nc.sync.dma_start(out=xb[:], in_=xt_re[:, 4:8, 0:512])
                        xt_parts.append([(xa, 0), (xb, 4)])
                        wv = cpool.tile([128, 8, GC], bf16, tag="wv")
                        nc.sync.dma_start(
                            out=wv[:],
                            in_=wv_d[:].rearrange("(c p) m -> p c m", p=128),
                        )
                    else:
                        x_s = cpool.tile([128, 8, 512], bf16, tag=f"xt{s}")
                        nc.sync.dma_start(
                            out=x_s[:],
                            in_=xt_re[:, :, s * 512 : (s + 1) * 512],
                        )
                        xt_parts.append([(x_s, 0)])

                def xslice(s, c, sl):
                    for t_, c0 in xt_parts[s]:
                        if c0 <= c < c0 + 4 or (c0 == 0 and len(xt_parts[s]) == 1):
                            return t_[:, c - c0, sl]
                    raise AssertionError
                # W_proj rows for the head pair j live at partitions
                # [0..127] = channels j*128..j*128+127
                wp = cpool.tile([128, 2, C], bf16, tag="wp")
                wp_ref[0] = wp
                nc.sync.dma_start(
                    out=wp[:], in_=wp_d[:].rearrange("(j p) n -> p j n", p=128)
                )

                # ones row at partition 64 for the denominator broadcast
                ones64 = cpool.tile([65, 64], f32, tag="ones64")
                nc.any.memset(ones64[64:65, :], 1.0)
                zbias = cpool.tile([128, 1], f32, tag="zbias")
                nc.any.memset(zbias[:], 0.0)

                # ---- QKV tiles + attention, interleaved per query super-tile
                # so ScalarE exp work starts as early as possible ----
                if fp8_sim:
                    # q,k quantized to fp8e4 in [128, slot, hh, T] layout for
                    # DoubleRow sim matmuls; slot 1 is all-zero padding (the
                    # pairing contributes k.q + 0.junk), so evictions stay
                    # one [128,512] copy per m-tile and no partition moves
                    # are needed.  Head h=2*hh+a lives at partitions 64a..64a+63.
                    qz8 = cpool.tile([128, 2, 2, T], f8, tag="qz8")
                    kz8 = cpool.tile([128, 2, 2, T], f8, tag="kz8")
                    nc.gpsimd.memset(qz8[:, 1, :, :], 0.0)
                    nc.gpsimd.memset(kz8[:, 1, :, :], 0.0)
                else:
                    qkT = cpool.tile([128, 4, T], bf16, tag="qkT")
                v1 = cpool.tile([128, NT, HPC, 65], bf16, tag="v1")
                nc.gpsimd.memset(v1[:, :, :, 64:65], 1.0)
                # y^T packed 2 heads per 128 partitions: [128, pair, T]
                yta = cpool.tile([128, 2, T], bf16, tag="yta")
                yta_ref[0] = yta

                for s in range(NS):
                    # q^T,k^T columns for this super-tile
                    for m in range(4):
                        ps = mmp.tile([128, 512], f32, tag="mm")
                        if fp8_qk:
                            for u in range(4):
                                nc.tensor.matmul(
                                    ps[:],
                                    wqk8[:, u, :, m * 128 : (m + 1) * 128],
                                    x8slice(s, u),
                                    start=(u == 0),
                                    stop=(u == 3),
                                    perf_mode=DR,
                                )
                        else:
                            for c in range(8):
                                nc.tensor.matmul(
                                    ps[:],
                                    wqk_slice(c, slice(m * 128, (m + 1) * 128)),
                                    xslice(s, c, slice(None)),
                                    start=(c == 0),
                                    stop=(c == 7),
                                )
                        if fp8_sim:
                            dst = qz8 if m < 2 else kz8
                            nc.vector.tensor_copy(
                                dst[:, 0, m % 2, s * 512 : (s + 1) * 512], ps[:]
                            )
                        else:
                            nc.vector.tensor_copy(
                                qkT[:, m, s * 512 : (s + 1) * 512], ps[:]
                            )
                    # v rows for this super-tile (+ ones column)
                    for t in range(s * 4, s * 4 + 4):
                        ps = mmp.tile([128, GC], f32, tag="mm")
                        for c in range(8):
                            nc.tensor.matmul(
                                ps[:],
                                xslice(
                                    s,
                                    c,
                                    slice((t - 4 * s) * 128, (t - 4 * s + 1) * 128),
                                ),
                                wv[:, c, :],
                                start=(c == 0),
                                stop=(c == 7),
                            )
                        nc.vector.tensor_copy(
                            v1[:, t, :, 0:64],
                            ps[:].rearrange("p (l d) -> p l d", d=64),
                        )

                    for li, l in enumerate((0, 1, 3, 2)):
                        if s > 0 and li in (1, 2):
                            # projection for the previous super-tile, deferred
                            # and split across two heads so its y^T inputs
                            # (incl. the odd-head SBUF DMA) have settled and
                            # the mm-psum pool isn't hammered all at once
                            emit_proj(s - 1, (0, 1) if li == 1 else (2, 3))
                        poff = (l % 2) * 64
                        qt = l // 2
                        if not fp8_sim:
                            q_ap = qkT[poff : poff + 64, qt, s * 512 : (s + 1) * 512]
                        njt = 4 * (s + 1)
                        yps = ypp.tile([65, 512], f32, tag="y")
                        for grp in range(njt // 2):
                            sp = simp_pool.tile([128, 1024], f32, tag="sim")
                            for jj in range(2):
                                j = grp * 2 + jj
                                r = j - 4 * s
                                # causal: diagonal tile j=4s+r only needs
                                # query columns >= r*128
                                q0 = r * 128 if r > 0 else 0
                                if fp8_sim:
                                    nc.tensor.matmul(
                                        sp[:, jj * 512 + q0 : (jj + 1) * 512],
                                        kz8[
                                            poff : poff + 64,
                                            :,
                                            qt,
                                            j * 128 : (j + 1) * 128,
                                        ],
                                        qz8[
                                            poff : poff + 64,
                                            :,
                                            qt,
                                            s * 512 + q0 : (s + 1) * 512,
                                        ],
                                        start=True,
                                        stop=True,
                                        perf_mode=DR,
                                    )
                                else:
                                    k_ap = qkT[
                                        poff : poff + 64,
                                        2 + qt,
                                        j * 128 : (j + 1) * 128,
                                    ]
                                    nc.tensor.matmul(
                                        sp[:, jj * 512 + q0 : (jj + 1) * 512],
                                        k_ap,
                                        q_ap[:, q0:],
                                        start=True,
                                        stop=True,
                                    )
                            ex = epool.tile([128, 1024], bf16, tag="exp")
                            if grp >= 2 * s:
                                # diagonal pair: exp only the causally valid
                                # column range of each tile
                                for jj in range(2):
                                    j = grp * 2 + jj
                                    r = j - 4 * s
                                    q0 = r * 128 if r > 0 else 0
                                    nc.scalar.activation(
                                        ex[:, jj * 512 + q0 : (jj + 1) * 512],
                                        sp[:, jj * 512 + q0 : (jj + 1) * 512],
                                        AF.Exp,
                                        bias=zbias[:, 0:1],
                                        scale=1.0 / 32.0,
                                    )
                            else:
                                nc.scalar.activation(
                                    ex[:],
                                    sp[:],
                                    AF.Exp,
                                    bias=zbias[:, 0:1],
                                    scale=1.0 / 32.0,
                                )
                            for jj in range(2):
                                j = grp * 2 + jj
                                r = j - 4 * s
                                q0 = r * 128 if r > 0 else 0
                                if 0 <= r < 4:
                                    # zero below-diagonal within the 128-col
                                    # diagonal block: keep where f' >= p.
                                    # Columns past the block are fully valid.
                                    nc.gpsimd.affine_select(
                                        out=ex[:, jj * 512 + q0 : jj * 512 + q0 + 128],
                                        in_=ex[:, jj * 512 + q0 : jj * 512 + q0 + 128],
                                        pattern=[[1, 128]],
                                        compare_op=mybir.AluOpType.is_ge,
                                        fill=0.0,
                                        base=0,
                                        channel_multiplier=-1,
                                    )
                                nc.tensor.matmul(
                                    yps[:, q0:],
                                    v1[:, j, l, :],
                                    ex[:, jj * 512 + q0 : (jj + 1) * 512],
                                    start=(j == 0),
                                    stop=(j == njt - 1),
                                    skip_group_check=True,
                                )
                        # normalize: row 64 of yps is the denominator
                        rt = spool.tile([65, 512], f32, tag="rt")
                        nc.vector.reciprocal(rt[64:65, :], yps[64:65, :])
                        if l % 2 == 0:
                            ysl = yta[0:64, qt, s * 512 : (s + 1) * 512]
                        else:
                            ytmp = spool.tile([64, 512], bf16, tag="ytmp")
                            ysl = ytmp[:]
                        # for the very last head the PE is idle anyway and
                        # the engine-local PE-broadcast chain avoids the DMA
                        # hop latency right before the tail projection
                        if use_pbcast and not (s == NS - 1 and li == 3):
                            # partition_broadcast on HW reads the tile's
                            # physical partition 0 - hop the denominator row
                            # down with a tiny SBUF->SBUF DMA first
                            rt0 = spool.tile([1, 512], f32, tag="rt0")
                            nc.sync.dma_start(out=rt0[:], in_=rt[64:65, :])
                            bps = spool.tile([64, 512], f32, tag="bps")
                            nc.gpsimd.partition_broadcast(bps[:], rt0[:])
                            nc.vector.tensor_mul(ysl, yps[0:64, :], bps[:])
                        else:
                            bp = mmp.tile([64, 512], f32, tag="mm")
                            nc.tensor.matmul(
                                bp[:],
                                ones64[64:65, :],
                                rt[64:65, :],
                                start=True,
                                stop=True,
                            )
                            nc.vector.tensor_copy(ysl, yps[0:64, :])
                            nc.vector.tensor_mul(ysl, ysl, bp[:])
                        if l % 2 == 1:
                            nc.sync.dma_start(
                                out=yta[64:128, qt, s * 512 : (s + 1) * 512],
                                in_=ysl,
                            )

                # last super-tile's projection runs at the tail
                emit_proj(NS - 1, (0, 1, 2, 3))

            def emit_proj(s, tts):
                for tt in tts:
                    t = s * 4 + tt
                    ost = opool.tile([128, C], bf16, tag="ost")
                    for n in range(2):
                        pp = mmp.tile([128, 512], f32, tag="mm")
                        for j in range(2):
                            nc.tensor.matmul(
                                pp[:],
                                yta_ref[0][:, j, t * 128 : (t + 1) * 128],
                                wp_ref[0][:, j, n * 512 : (n + 1) * 512],
                                start=(j == 0),
                                stop=(j == 1),
                            )
                        nc.vector.tensor_copy(ost[:, n * 512 : (n + 1) * 512], pp[:])
                    nc.sync.dma_start(
                        out=out_d[t * 128 : (t + 1) * 128, :], in_=ost[:]
                    )

            for _rep in range(repeat):
                emit_once()

    nc.compile()
    return nc


def _get_nc():
    global _cached
    if _cached is None:
        _cached = _build()
    return _cached


def build_in_maps(inputs):
    # b_attn is spec-guaranteed zeros and is not used on-device; b_proj is
    # added on host.
    x = np.asarray(inputs["x"], dtype=np.float32)
    W_attn = np.asarray(inputs["W_attn"], dtype=np.float32)
    W_proj = np.asarray(inputs["W_proj"], dtype=np.float32)

    F8 = ml_dtypes.float8_e4m3
    in_maps = []
    for b in range(B):
        xT = np.ascontiguousarray(x[b].T).astype(BF16)
        xT8 = xT.astype(F8) if FP8_QK else None
        for g in range(4):
            c0 = g * GC
            wq = W_attn[:, c0 : c0 + GC]
            wk = W_attn[:, C + c0 : C + c0 + GC]
            wqk = np.ascontiguousarray(np.concatenate([wq, wk], axis=1)).astype(BF16)
            wv = np.ascontiguousarray(
                W_attn[:, 2 * C + c0 : 2 * C + c0 + GC]
            ).astype(BF16)
            wp = np.ascontiguousarray(W_proj[c0 : c0 + GC, :]).astype(BF16)
            m = {"xt": xT, "wv": wv, "wp": wp}
            if FP8_QK:
                m["xt8"] = xT8
                m["wqk8"] = wqk.astype(F8)
            else:
                m["wqk"] = wqk
            in_maps.append(m)
    return in_maps


def kernel(x, W_attn, b_attn, W_proj, b_proj):
    from concourse.bass_utils import run_bass_kernel_spmd

    b_proj = np.asarray(b_proj, dtype=np.float32)
    nc = _get_nc()
    in_maps = build_in_maps(
        {"x": x, "W_attn": W_attn, "b_attn": b_attn, "W_proj": W_proj}
    )
    res = run_bass_kernel_spmd(nc, in_maps, core_ids=list(range(8)))
    out = np.zeros((B, T, C), dtype=np.float32)
    for b in range(B):
        for g in range(4):
            out[b] += res.results[b * 4 + g]["out"].astype(np.float32)
        out[b] += b_proj
    return out



# revision 36
# speedup vs baseline: 15.4716x; 1.4003x over previous
"""Causal self-attention (B=2, T=2048, C=1024, H=16) on 8 TRN2 NeuronCores.

Sharding: core = b*4 + g  (b in 0..1 batches, g in 0..3 head-groups of 4 heads).
Each core computes QKV for its 4 heads (tensor-parallel columns of W_attn),
full causal attention over T=2048, and a partial projection
y_g @ W_proj[rows_g] -> [T, C].  Host sums the 4 partials per batch and adds
b_proj.

Device layout notes:
  - x is pre-transposed on host to xT [C, T] so all matmuls contract over
    partitions.
  - sim is computed transposed: simT[tk, tq] = k . q   (lhsT=kT, rhs=qT),
    exp via ScalarE (scale=1/sqrt(C) folded in), causal mask applied as a
    gpsimd affine_select (exact zeros) on the exp tiles.
  - attn@v: lhsT = [v | ones] per head -> psum rows 0..63 = unnormalized y^T,
    row 64 = softmax denominator.  DVE reciprocal -> tiny SBUF DMA to
    partition 0 -> gpsimd partition_broadcast yields a [64, 512] SBUF tile
    aligned with y^T; one DVE multiply evicts normalized y^T to SBUF bf16.
    (PE-broadcast fallback kept under use_pbcast=False.)
  - y^T is packed two heads per 128 partitions (odd heads placed via a small
    SBUF->SBUF DMA) so the projection contracts K=128.
  - all matmul inputs bf16 (host-cast), accumulation f32, partial outputs
    bf16 (summed in f32 on host).
"""

import sys

sys.path.insert(0, "/opt/trn_rl_repo")

import numpy as np
import ml_dtypes

BF16 = ml_dtypes.bfloat16

B, T, C = 2, 2048, 1024
H, D = 16, 64
HPC = 4          # heads per core
GC = HPC * D     # head-group channel width (256)
NT = T // 128    # 16 row tiles
NS = T // 512    # 4 query super-tiles

FP8_QK = True    # q,k QKV matmuls in fp8e4 DoubleRow (paired c-tiles);
                 # measured 105ns vs 2x166ns bf16 per 512-col instruction
QK_RESID = True  # add a W-residual second pass (w ~ w8 + wr8) to cut the
                 # fp8 weight-quantization error
# sim note: K=64 matmuls measured 416ns vs K=128's 166ns per 512 cols, so the
# sim runs block-diagonally at K=128: lhsT = packed k pair (junk rows OK),
# rhs = q in a per-head slot whose other 64 partitions are zero - exact math.

_cached = None


def _build(repeat=1, use_pbcast=True, fp8_qk=FP8_QK, qk_resid=QK_RESID):
    # note: tensor_mul with both operands in PSUM is rejected by the walrus
    # BIR verifier, so the non-pbcast path uses a two-step evict.
    import concourse.bass as bass  # noqa: F401
    import concourse.mybir as mybir
    import concourse.tile as tile
    from concourse import bacc

    f32 = mybir.dt.float32
    bf16 = mybir.dt.bfloat16
    f8 = mybir.dt.float8e4
    AF = mybir.ActivationFunctionType
    DR = mybir.MatmulPerfMode.DoubleRow

    nc = bacc.Bacc(None, target_bir_lowering=False, debug=False)
    xt_d = nc.declare_dram_parameter("xt", [C, T], bf16, isOutput=False)
    if fp8_qk:
        xt8_d = nc.declare_dram_parameter("xt8", [C, T], f8, isOutput=False)
        wqk8_d = nc.declare_dram_parameter(
            "wqk8", [C, 2 * GC], f8, isOutput=False
        )
        if qk_resid:
            wqr8_d = nc.declare_dram_parameter(
                "wqr8", [C, 2 * GC], f8, isOutput=False
            )
    else:
        wqk_d = nc.declare_dram_parameter(
            "wqk", [C, 2 * GC], bf16, isOutput=False
        )
    wv_d = nc.declare_dram_parameter("wv", [C, GC], bf16, isOutput=False)
    wp_d = nc.declare_dram_parameter("wp", [GC, C], bf16, isOutput=False)
    out_d = nc.declare_dram_parameter("out", [T, C], bf16, isOutput=True)

    with tile.TileContext(nc) as tc:
        with (
            tc.tile_pool(name="const", bufs=1) as cpool,
            tc.tile_pool(name="exp", bufs=4) as epool,
            tc.tile_pool(name="ostg", bufs=2) as opool,
            tc.tile_pool(name="smal", bufs=3) as spool,
            tc.tile_pool(name="mm", bufs=2, space="PSUM") as mmp,
            tc.tile_pool(name="sim", bufs=2, space="PSUM") as simp_pool,
            tc.tile_pool(name="yp", bufs=2, space="PSUM") as ypp,
        ):
            yta_ref = [None]
            wp_ref = [None]

            def emit_once():
                # ---- inputs: weights first (small), then xT spread over
                # several DMAs ----
                if fp8_qk:
                    # paired-c-tile fp8 layout for DoubleRow: [p, u, i, cols],
                    # contraction c = (2u+i)*128 + p
                    wqk8 = cpool.tile([128, 4, 2, 2 * GC], f8, tag="wqk8")
                    nc.sync.dma_start(
                        out=wqk8[:],
                        in_=wqk8_d[:].rearrange(
                            "(u i p) m -> p u i m", p=128, i=2
                        ),
                    )
                    if qk_resid:
                        wqr8 = cpool.tile([128, 4, 2, 2 * GC], f8, tag="wqr8")
                        nc.sync.dma_start(
                            out=wqr8[:],
                            in_=wqr8_d[:].rearrange(
                                "(u i p) m -> p u i m", p=128, i=2
                            ),
                        )
                    xt8_re = xt8_d[:].rearrange("(u i p) t -> p u i t", p=128, i=2)
                    x8a = cpool.tile([128, 2, 2, 512], f8, tag="x8_0a")
                    nc.sync.dma_start(out=x8a[:], in_=xt8_re[:, 0:2, :, 0:512])
                    x8b = cpool.tile([128, 2, 2, 512], f8, tag="x8_0b")
                    nc.sync.dma_start(out=x8b[:], in_=xt8_re[:, 2:4, :, 0:512])
                    x8_parts = [[(x8a, 0), (x8b, 2)]]
                    for s in range(1, NS):
                        x8s = cpool.tile([128, 4, 2, 512], f8, tag=f"x8_{s}")
                        nc.sync.dma_start(
                            out=x8s[:],
                            in_=xt8_re[:, :, :, s * 512 : (s + 1) * 512],
                        )
                        x8_parts.append([(x8s, 0)])

                    def x8slice(s, u):
                        for t_, u0 in x8_parts[s]:
                            if u0 <= u < u0 + 2 or (
                                u0 == 0 and len(x8_parts[s]) == 1
                            ):
                                return t_[:, u - u0, :, :]
                        raise AssertionError
                else:
                    wqk_re = wqk_d[:].rearrange("(c p) m -> p c m", p=128)
                    wqka = cpool.tile([128, 4, 2 * GC], bf16, tag="wqka")
                    nc.sync.dma_start(out=wqka[:], in_=wqk_re[:, 0:4, :])
                    wqkb = cpool.tile([128, 4, 2 * GC], bf16, tag="wqkb")
                    nc.sync.dma_start(out=wqkb[:], in_=wqk_re[:, 4:8, :])

                    def wqk_slice(c, sl):
                        t_ = wqka if c < 4 else wqkb
                        return t_[:, c % 4, sl]
                # x^T split per query super-tile so s=0 compute starts after
                # ~1MB of DMA instead of the full 4MB; the first chunk is
                # further halved so the very first matmuls start sooner
                xt_parts = []
                xt_re = xt_d[:].rearrange("(c p) t -> p c t", p=128)
                for s in range(NS):
                    if s == 0:
                        xa = cpool.tile([128, 4, 512], bf16, tag="xt0a")
                        nc.sync.dma_start(out=xa[:], in_=xt_re[:, 0:4, 0:512])
                        xb = cpool.tile([128, 4, 512], bf16, tag="xt0b")
                        nc.sync.dma_start(out=xb[:], in_=xt_re[:, 4:8, 0:512])
                        xt_parts.append([(xa, 0), (xb, 4)])
                        wv = cpool.tile([128, 8, GC], bf16, tag="wv")
                        nc.sync.dma_start(
                            out=wv[:],
                            in_=wv_d[:].rearrange("(c p) m -> p c m", p=128),
                        )
                    else:
                        x_s = cpool.tile([128, 8, 512], bf16, tag=f"xt{s}")
                        nc.sync.dma_start(
                            out=x_s[:],
                            in_=xt_re[:, :, s * 512 : (s + 1) * 512],
                        )
                        xt_parts.append([(x_s, 0)])

                def xslice(s, c, sl):
                    for t_, c0 in xt_parts[s]:
                        if c0 <= c < c0 + 4 or (c0 == 0 and len(xt_parts[s]) == 1):
                            return t_[:, c - c0, sl]
                    raise AssertionError
                # W_proj rows for the head pair j live at partitions
                # [0..127] = channels j*128..j*128+127
                wp = cpool.tile([128, 2, C], bf16, tag="wp")
                wp_ref[0] = wp
                nc.sync.dma_start(
                    out=wp[:], in_=wp_d[:].rearrange("(j p) n -> p j n", p=128)
                )

                # ones row at partition 64 for the denominator broadcast
                ones64 = cpool.tile([65, 64], f32, tag="ones64")
                nc.any.memset(ones64[64:65, :], 1.0)
                zbias = cpool.tile([128, 1], f32, tag="zbias")
                nc.any.memset(zbias[:], 0.0)

                # ---- QKV tiles + attention, interleaved per query super-tile
                # so ScalarE exp work starts as early as possible ----
                # q lives in per-head slots with the other head's 64
                # partitions zeroed, so the sim can contract a full K=128
                # (packed k pair as lhsT; its other-head rows are wiped out
                # by q's zeros)
                qzT = cpool.tile([128, HPC, T], bf16, tag="qzT")
                nc.gpsimd.memset(qzT[64:128, 0, :], 0.0)
                nc.gpsimd.memset(qzT[0:64, 1, :], 0.0)
                nc.gpsimd.memset(qzT[64:128, 2, :], 0.0)
                nc.gpsimd.memset(qzT[0:64, 3, :], 0.0)
                kT = cpool.tile([128, 2, T], bf16, tag="kT")
                v1 = cpool.tile([128, NT, HPC, 65], bf16, tag="v1")
                nc.gpsimd.memset(v1[:, :, :, 64:65], 1.0)
                # y^T packed 2 heads per 128 partitions: [128, pair, T]
                yta = cpool.tile([128, 2, T], bf16, tag="yta")
                yta_ref[0] = yta

                def emit_qk(s, ms):
                    for m in ms:
                        ps = mmp.tile([128, 512], f32, tag="mm", name="psqk")
                        if fp8_qk:
                            passes = (wqk8, wqr8) if qk_resid else (wqk8,)
                            for pi, wt in enumerate(passes):
                                for u in range(4):
                                    nc.tensor.matmul(
                                        ps[:],
                                        wt[:, u, :, m * 128 : (m + 1) * 128],
                                        x8slice(s, u),
                                        start=(pi == 0 and u == 0),
                                        stop=(
                                            pi == len(passes) - 1 and u == 3
                                        ),
                                        perf_mode=DR,
                                    )
                        else:
                            for c in range(8):
                                nc.tensor.matmul(
                                    ps[:],
                                    wqk_slice(c, slice(m * 128, (m + 1) * 128)),
                                    xslice(s, c, slice(None)),
                                    start=(c == 0),
                                    stop=(c == 7),
                                )
                        if m < 2:
                            # q: split the two heads into half-zero slots
                            nc.vector.tensor_copy(
                                qzT[0:64, 2 * m, s * 512 : (s + 1) * 512],
                                ps[0:64, :],
                            )
                            nc.vector.tensor_copy(
                                qzT[
                                    64:128, 2 * m + 1, s * 512 : (s + 1) * 512
                                ],
                                ps[64:128, :],
                            )
                        else:
                            nc.vector.tensor_copy(
                                kT[:, m - 2, s * 512 : (s + 1) * 512], ps[:]
                            )

                def emit_v(s, ts):
                    # v rows (+ ones column at col 64, preset once)
                    for t in ts:
                        ps = mmp.tile([128, GC], f32, tag="mm", name="psv")
                        for c in range(8):
                            nc.tensor.matmul(
                                ps[:],
                                xslice(
                                    s,
                                    c,
                                    slice((t - 4 * s) * 128, (t - 4 * s + 1) * 128),
                                ),
                                wv[:, c, :],
                                start=(c == 0),
                                stop=(c == 7),
                            )
                        nc.vector.tensor_copy(
                            v1[:, t, :, 0:64],
                            ps[:].rearrange("p (l d) -> p l d", d=64),
                        )

                def head_sims(s, l, grp):
                    # sim matmuls + exp + causal mask for one 2-key-tile
                    # group; returns the exp tile for the later attn@v.
                    # Full K=128 contraction: the other head's k rows are
                    # multiplied by q's zeroed partitions (exact).
                    qt = l // 2
                    sp = simp_pool.tile([128, 1024], f32, tag="sim", name="sp")
                    for jj in range(2):
                        j = grp * 2 + jj
                        r = j - 4 * s
                        # causal: diagonal tile j=4s+r only needs query
                        # columns >= r*128
                        q0 = r * 128 if r > 0 else 0
                        nc.tensor.matmul(
                            sp[:, jj * 512 + q0 : (jj + 1) * 512],
                            kT[:, qt, j * 128 : (j + 1) * 128],
                            qzT[:, l, s * 512 + q0 : (s + 1) * 512],
                            start=True,
                            stop=True,
                        )
                    ex = epool.tile([128, 1024], bf16, tag="exp", name="ex")
                    if grp >= 2 * s:
                        # diagonal pair: exp only the causally valid range
                        for jj in range(2):
                            j = grp * 2 + jj
                            r = j - 4 * s
                            q0 = r * 128 if r > 0 else 0
                            nc.scalar.activation(
                                ex[:, jj * 512 + q0 : (jj + 1) * 512],
                                sp[:, jj * 512 + q0 : (jj + 1) * 512],
                                AF.Exp,
                                bias=zbias[:, 0:1],
                                scale=1.0 / 32.0,
                            )
                            if 0 <= r < 4:
                                # zero below-diagonal within the 128-col
                                # diagonal block: keep where f' >= p
                                nc.gpsimd.affine_select(
                                    out=ex[
                                        :, jj * 512 + q0 : jj * 512 + q0 + 128
                                    ],
                                    in_=ex[
                                        :, jj * 512 + q0 : jj * 512 + q0 + 128
                                    ],
                                    pattern=[[1, 128]],
                                    compare_op=mybir.AluOpType.is_ge,
                                    fill=0.0,
                                    base=0,
                                    channel_multiplier=-1,
                                )
                    else:
                        nc.scalar.activation(
                            ex[:],
                            sp[:],
                            AF.Exp,
                            bias=zbias[:, 0:1],
                            scale=1.0 / 32.0,
                        )
                    return ex

                def head_avs(s, l, grp, ex, yps, njt):
                    for jj in range(2):
                        j = grp * 2 + jj
                        r = j - 4 * s
                        q0 = r * 128 if r > 0 else 0
                        nc.tensor.matmul(
                            yps[:, q0:],
                            v1[:, j, l, :],
                            ex[:, jj * 512 + q0 : (jj + 1) * 512],
                            start=(j == 0),
                            stop=(j == njt - 1),
                            skip_group_check=True,
                        )

                def head_norm(s, li, l, yps):
                    # normalize: row 64 of yps is the denominator
                    qt = l // 2
                    rt = spool.tile([65, 512], f32, tag="rt", name="rt")
                    nc.vector.reciprocal(rt[64:65, :], yps[64:65, :])
                    if l % 2 == 0:
                        ysl = yta[0:64, qt, s * 512 : (s + 1) * 512]
                    else:
                        ytmp = spool.tile([64, 512], bf16, tag="ytmp", name="yt")
                        ysl = ytmp[:]
                    # for the very last head the PE is idle anyway and the
                    # engine-local PE-broadcast chain avoids the DMA hop
                    # latency right before the tail projection
                    if use_pbcast and not (s == NS - 1 and li == 3):
                        # partition_broadcast on HW reads the tile's physical
                        # partition 0 - hop the denominator row down with a
                        # tiny SBUF->SBUF DMA first
                        rt0 = spool.tile([1, 512], f32, tag="rt0", name="rt0")
                        nc.sync.dma_start(out=rt0[:], in_=rt[64:65, :])
                        bps = spool.tile([64, 512], f32, tag="bps", name="bps")
                        nc.gpsimd.partition_broadcast(bps[:], rt0[:])
                        nc.vector.tensor_mul(ysl, yps[0:64, :], bps[:])
                    else:
                        bp = mmp.tile([64, 512], f32, tag="mm", name="bp")
                        nc.tensor.matmul(
                            bp[:],
                            ones64[64:65, :],
                            rt[64:65, :],
                            start=True,
                            stop=True,
                        )
                        nc.vector.tensor_copy(ysl, yps[0:64, :])
                        nc.vector.tensor_mul(ysl, ysl, bp[:])
                    if l % 2 == 1:
                        nc.sync.dma_start(
                            out=yta[64:128, qt, s * 512 : (s + 1) * 512],
                            in_=ysl,
                        )

                # ---- s=0 prefix: heads 0,1's q/k first, then head 0's sims
                # and exps immediately so ScalarE starts ~3 chains in; v and
                # the other heads' q/k overlap those exps ----
                emit_qk(0, (0, 2))
                yps0 = ypp.tile([65, 512], f32, tag="y", name="yps0")
                ex00 = head_sims(0, 0, 0)
                ex01 = head_sims(0, 0, 1)
                emit_v(0, (0, 1))
                emit_v(0, (2, 3))
                emit_qk(0, (1, 3))
                head_avs(0, 0, 0, ex00, yps0, 4)
                head_avs(0, 0, 1, ex01, yps0, 4)
                head_norm(0, 0, 0, yps0)

                for s in range(NS):
                    # filler PE work (next super-tile's QKV, previous
                    # super-tile's projection), popped into the gaps where
                    # the PE would otherwise stall waiting on exp
                    fillers = []
                    if s < NS - 1:
                        fillers.append(lambda s=s: emit_qk(s + 1, (0,)))
                        fillers.append(lambda s=s: emit_qk(s + 1, (2,)))
                    if s > 0:
                        fillers.append(lambda s=s: emit_proj(s - 1, (0,)))
                        fillers.append(lambda s=s: emit_proj(s - 1, (1,)))
                    if s < NS - 1:
                        fillers.append(lambda s=s: emit_v(s + 1, (4 * s + 4,)))
                        fillers.append(lambda s=s: emit_v(s + 1, (4 * s + 5,)))
                        fillers.append(lambda s=s: emit_qk(s + 1, (1,)))
                    if s > 0:
                        fillers.append(lambda s=s: emit_proj(s - 1, (2,)))
                        fillers.append(lambda s=s: emit_proj(s - 1, (3,)))
                    if s < NS - 1:
                        fillers.append(lambda s=s: emit_qk(s + 1, (3,)))
                        fillers.append(lambda s=s: emit_v(s + 1, (4 * s + 6,)))
                        fillers.append(lambda s=s: emit_v(s + 1, (4 * s + 7,)))

                    njt = 4 * (s + 1)
                    G = njt // 2
                    heads = (1, 3, 2) if s == 0 else (0, 1, 3, 2)
                    gaps_total = len(heads) * (G + 1)
                    gap_i = 0
                    popped = 0

                    def pop_filler():
                        nonlocal gap_i, popped
                        gap_i += 1
                        while fillers and popped < (
                            gap_i * len(fillers_all)
                        ) // gaps_total:
                            fillers.pop(0)()
                            popped += 1

                    fillers_all = list(fillers)
                    for li, l in enumerate(heads):
                        if s == 0:
                            li += 1  # head 0 was handled in the prefix
                        yps = ypp.tile([65, 512], f32, tag="y", name="yps")
                        pend = None
                        for grp in range(G):
                            ex = head_sims(s, l, grp)
                            pop_filler()
                            if pend is not None:
                                head_avs(s, l, pend[0], pend[1], yps, njt)
                            pend = (grp, ex)
                        head_avs(s, l, pend[0], pend[1], yps, njt)
                        head_norm(s, li, l, yps)
                        pop_filler()

                # last super-tile's projection runs at the tail
                emit_proj(NS - 1, (0, 1, 2, 3))

            def emit_proj(s, tts):
                for tt in tts:
                    t = s * 4 + tt
                    ost = opool.tile([128, C], bf16, tag="ost")
                    for n in range(2):
                        pp = mmp.tile([128, 512], f32, tag="mm")
                        for j in range(2):
                            nc.tensor.matmul(
                                pp[:],
                                yta_ref[0][:, j, t * 128 : (t + 1) * 128],
                                wp_ref[0][:, j, n * 512 : (n + 1) * 512],
                                start=(j == 0),
                                stop=(j == 1),
                            )
                        nc.vector.tensor_copy(ost[:, n * 512 : (n + 1) * 512], pp[:])
                    nc.sync.dma_start(
                        out=out_d[t * 128 : (t + 1) * 128, :], in_=ost[:]
                    )

            for _rep in range(repeat):
                emit_once()

    nc.compile()
    return nc


def _get_nc():
    global _cached
    if _cached is None:
        _cached = _build()
    return _cached


def build_in_maps(inputs):
    # b_attn is spec-guaranteed zeros and is not used on-device; b_proj is
    # added on host.
    x = np.asarray(inputs["x"], dtype=np.float32)
    W_attn = np.asarray(inputs["W_attn"], dtype=np.float32)
    W_proj = np.asarray(inputs["W_proj"], dtype=np.float32)

    F8 = ml_dtypes.float8_e4m3
    in_maps = []
    for b in range(B):
        xT = np.ascontiguousarray(x[b].T).astype(BF16)
        xT8 = xT.astype(F8) if FP8_QK else None
        for g in range(4):
            c0 = g * GC
            wq = W_attn[:, c0 : c0 + GC]
            wk = W_attn[:, C + c0 : C + c0 + GC]
            wqk = np.ascontiguousarray(np.concatenate([wq, wk], axis=1)).astype(BF16)
            wv = np.ascontiguousarray(
                W_attn[:, 2 * C + c0 : 2 * C + c0 + GC]
            ).astype(BF16)
            wp = np.ascontiguousarray(W_proj[c0 : c0 + GC, :]).astype(BF16)
            m = {"xt": xT, "wv": wv, "wp": wp}
            if FP8_QK:
                m["xt8"] = xT8
                w8 = wqk.astype(F8)
                m["wqk8"] = w8
                if QK_RESID:
                    m["wqr8"] = (
                        wqk.astype(np.float32) - w8.astype(np.float32)
                    ).astype(F8)
            else:
                m["wqk"] = wqk
            in_maps.append(m)
    return in_maps


def kernel(x, W_attn, b_attn, W_proj, b_proj):
    from concourse.bass_utils import run_bass_kernel_spmd

    b_proj = np.asarray(b_proj, dtype=np.float32)
    nc = _get_nc()
    in_maps = build_in_maps(
        {"x": x, "W_attn": W_attn, "b_attn": b_attn, "W_proj": W_proj}
    )
    res = run_bass_kernel_spmd(nc, in_maps, core_ids=list(range(8)))
    out = np.zeros((B, T, C), dtype=np.float32)
    for b in range(B):
        for g in range(4):
            out[b] += res.results[b * 4 + g]["out"].astype(np.float32)
        out[b] += b_proj
    return out

